# revision 11
# baseline (speedup 1.0000x reference)
"""AnchorTargetLayer max-IoU kernel for 8 TRN2 NeuronCores (v4, fp16).

max_iou[b, n] = max_g IoU(anchor_n, gt_box[b, g]);
anchors [100000, 4] f32, gt_boxes [4, 64, 4] f32 -> out [4, 100000] f32.

Sharding: anchors split 8 ways (12544/core incl pad), gt replicated, no
collectives. Per-core layout: anchors on SBUF partitions (128/block, 98
blocks), all B*G = 256 (batch, gt) pairs on the free dim, batch-major.

Coordinates are pre-scaled by 1/16 on the host and GT rows cast to fp16
(anchor per-partition scalars stay f32 as the ISA requires), keeping
every intermediate in fp16 range. Chain error vs f64 ref ~3e-3.

Measured DVE atoms (fp16, 256-wide): tensor_scalar w/ SBUF scalar runs
in 4x perf mode (~276ns) but scalar_tensor_tensor has NO fast uop
(~473ns). So the front uses only TS + one wide TT:
  t  = min(grow2, a2)     4 independent TS-ptr ops per block (x,y axes)
  m  = min(grow1n, na1)
  s  = t + m              ONE wide 2x tensor_tensor per superblock
                          (= iw|ih for both axes)
  int = relu(s_x) * s_y   Pool STT (s_y relu deferred to final clamp)
  sa  = garea + areaA     ACT Identity+bias (narrow)
  rs  = 1/sa              ACT Reciprocal (wide)
  w   = int * rs          Pool tensor_tensor (wide)
  red: max over g=64 -> two pairwise-max TTs on Pool (64->32->16), then
       a DVE tensor_reduce over the last 16.
Final per anchor: v = relu(vout); iou = v / (1 - v)  (negative w from
the deferred relu can only lower the max; clamped here).

Superblocks of C=7 blocks double-buffered (NB=2) across DVE -> Pool.
"""

import os
import sys

import numpy as np

sys.path.insert(0, "/opt/trn_rl_repo")

import concourse.bass as bass
import concourse.mybir as mybir
from concourse.bass_utils import run_bass_kernel_spmd

N_ANCHORS = 100000
BATCH = 4
N_GT = 64
N_CORES = 8

P = 128
BLOCKS = 98
C = 7                       # blocks per superblock
NSB = BLOCKS // C           # 14 superblocks
N_LOC = P * BLOCKS          # 12544
N_PAD = N_LOC * N_CORES     # 100352
NPAIR = BATCH * N_GT        # 256
SEG = C * BATCH             # 28 (block, batch) rows per superblock
NB = 2                      # superblock double-buffer depth

F32 = mybir.dt.float32
F16 = mybir.dt.float16
COORD_SCALE = 1.0 / 16.0

LAST_EXEC_NS = None


def _ensure_axon_ntff_hook():
    try:
        import antenv.axon_hooks  # noqa: F401

        return
    except ImportError:
        pass
    import contextlib
    import ctypes
    import types

    import antenv

    m = types.ModuleType("antenv.axon_hooks")
    m._hook = None

    def set_axon_ntff_profile_hook(h):
        m._hook = h

    def get_axon_ntff_profile_hook():
        return m._hook

    m.set_axon_ntff_profile_hook = set_axon_ntff_profile_hook
    m.get_axon_ntff_profile_hook = get_axon_ntff_profile_hook
    sys.modules["antenv.axon_hooks"] = m
    antenv.axon_hooks = m

    so_path = os.environ.get("PJRT_LIBRARY_PATH", "/opt/axon/libaxon_pjrt.so")
    try:
        lib = ctypes.CDLL(so_path)
    except OSError:
        return
    if not hasattr(lib, "axon_start_nrt_profile"):
        return
    lib.axon_start_nrt_profile.argtypes = [
        ctypes.POINTER(ctypes.c_int64),
        ctypes.c_size_t,
    ]
    lib.axon_start_nrt_profile.restype = ctypes.c_int64
    lib.axon_stop_nrt_profile.argtypes = [ctypes.c_char_p]
    lib.axon_stop_nrt_profile.restype = ctypes.c_int64

    @contextlib.contextmanager
    def _hook(output_dir, device_ids):
        import jax

        jax.devices()
        if device_ids:
            ids = (ctypes.c_int64 * len(device_ids))(*device_ids)
            rc = lib.axon_start_nrt_profile(ids, len(device_ids))
        else:
            rc = lib.axon_start_nrt_profile(None, 0)
        if rc != 0:
            raise RuntimeError(f"axon_start_nrt_profile rc={rc}")
        try:
            yield
        finally:
            n = lib.axon_stop_nrt_profile(str(output_dir).encode())
            if n < 0:
                raise RuntimeError(f"axon_stop_nrt_profile rc={n}")

    set_axon_ntff_profile_hook(_hook)


def _patch_upload_artifacts():
    import concourse.bass_utils as bu

    if getattr(bu.upload_artifacts, "_safe", False):
        return
    orig = bu.upload_artifacts

    def safe(tmpdir):
        try:
            return orig(tmpdir)
        except Exception:
            return tmpdir

    safe._safe = True
    bu.upload_artifacts = safe


def _act_recip(scalar_eng, nc, out_ap, in_ap, bias=0.0, scale=1.0):
    """Directly emit Activation(Reciprocal) (the nc.scalar.activation wrapper
    rejects Reciprocal)."""
    ins = [scalar_eng.lower_ap(in_ap)]
    for argv in (bias, scale, 0.0):  # bias, scale, alpha
        ins.append(mybir.ImmediateValue(dtype=F32, value=argv))
    return scalar_eng.add_instruction(
        mybir.InstActivation(
            name=nc.get_next_instruction_name(),
            func=mybir.ActivationFunctionType.Reciprocal,
            ins=ins,
            outs=[scalar_eng.lower_ap(out_ap)],
        )
    )


def _build_graph():
    nc = bass.Bass()
    ASC_ext = nc.declare_dram_parameter("ascal", [P, BLOCKS * 4], F32, isOutput=False)
    AR_ext = nc.declare_dram_parameter("aarea", [P, BLOCKS], F32, isOutput=False)
    GT_ext = nc.declare_dram_parameter("gtrows", [5, NPAIR], F16, isOutput=False)
    out_ext = nc.declare_dram_parameter("out", [P, BLOCKS * 4], F32, isOutput=True)

    Alu = mybir.AluOpType

    from contextlib import ExitStack

    with ExitStack() as _st:
        e = _st.enter_context
        ASC = e(nc.sbuf_tensor("ASC", [P, BLOCKS * 4], F32))
        AR = e(nc.sbuf_tensor("AR", [P, BLOCKS], F32))
        GTB = e(nc.sbuf_tensor("GTB", [P, 5, NPAIR], F16))
        T = e(nc.sbuf_tensor("T", [P, NB, 2, C, NPAIR], F16))
        M = e(nc.sbuf_tensor("M", [P, NB, 2, C, NPAIR], F16))
        S = e(nc.sbuf_tensor("S", [P, NB, 2, C, NPAIR], F16))
        SXR = e(nc.sbuf_tensor("SXR", [P, NB, C, NPAIR], F16))
        INT = e(nc.sbuf_tensor("INT", [P, NB, C, NPAIR], F16))
        SA = e(nc.sbuf_tensor("SA", [P, NB, C, NPAIR], F16))
        RS = e(nc.sbuf_tensor("RS", [P, NB, C, NPAIR], F16))
        W = e(nc.sbuf_tensor("W", [P, 3, SEG, N_GT], F16))
        M1 = e(nc.sbuf_tensor("M1", [P, 3, SEG, 32], F16))
        M2 = e(nc.sbuf_tensor("M2", [P, 3, SEG, 16], F16))
        VOUT = e(nc.sbuf_tensor("VOUT", [P, BLOCKS * 4], F32))
        V2 = e(nc.sbuf_tensor("V2", [P, BLOCKS * 4], F32))
        R1 = e(nc.sbuf_tensor("R1", [P, BLOCKS * 4], F32))
        MIOU = e(nc.sbuf_tensor("MIOU", [P, BLOCKS * 4], F32))
        block = e(nc.Block())
        dma_sem = e(nc.semaphore("dma_sem"))
        dve_sem = e(nc.semaphore("dve_sem"))
        act_sem = e(nc.semaphore("act_sem"))
        pool_sem = e(nc.semaphore("pool_sem"))
        GX2 = GTB[:, 0, :]
        GX1N = GTB[:, 1, :]
        GY2 = GTB[:, 2, :]
        GY1N = GTB[:, 3, :]
        GAREA = GTB[:, 4, :]

        @block.sync
        def _(sync):
            sync.dma_start(out=ASC[:, :], in_=ASC_ext[:, :]).then_inc(dma_sem, 16)
            sync.dma_start(out=AR[:, :], in_=AR_ext[:, :]).then_inc(dma_sem, 16)
            g_ap = GT_ext[:, :]
            g_b = bass.AP(
                tensor=g_ap.tensor, offset=g_ap.offset, ap=[[0, P]] + list(g_ap.ap)
            )
            sync.dma_start(out=GTB[:, :, :], in_=g_b).then_inc(dma_sem, 16)

        # sem targets: dve: s+1 after S(s); NSB+1 v2; NSB+2 miou
        #              act: 2s+1 SXR(s); 2s+2 rs(s); 2NSB+1 r1
        #              pool: 2s+1 int(s); 2s+2 w(s)
        def dve_mchain(vector, s):
            # pairwise-max 64->32->16 then reduce over the last 16
            vector.wait_ge(pool_sem, 2 * s + 2)
            sb = s % 3
            wv = W[:, sb, :, :]
            vector.tensor_tensor(
                out=M1[:, sb, :, :], in0=wv[:, :, 0:32], in1=wv[:, :, 32:64],
                op=Alu.max,
            )
            vector.tensor_tensor(
                out=M2[:, sb, :, :],
                in0=M1[:, sb, :, 0:16], in1=M1[:, sb, :, 16:32],
                op=Alu.max,
            )
            vector.tensor_reduce(
                out=VOUT[:, s * SEG : (s + 1) * SEG],
                in_=M2[:, sb, :, :],
                axis=mybir.AxisListType.X,
                op=Alu.max,
            )

        @block.vector
        def _(vector):
            vector.wait_ge(dma_sem, 48)
            for s in range(NSB):
                sb = s % NB
                for j in range(C):
                    blk = s * C + j
                    ax2 = ASC[:, 4 * blk + 0 : 4 * blk + 1]
                    nax1 = ASC[:, 4 * blk + 1 : 4 * blk + 2]
                    ay2 = ASC[:, 4 * blk + 2 : 4 * blk + 3]
                    nay1 = ASC[:, 4 * blk + 3 : 4 * blk + 4]
                    vector.tensor_scalar(
                        out=T[:, sb, 0, j, :], in0=GX2, scalar1=ax2,
                        scalar2=None, op0=Alu.min,
                    )
                    vector.tensor_scalar(
                        out=T[:, sb, 1, j, :], in0=GY2, scalar1=ay2,
                        scalar2=None, op0=Alu.min,
                    )
                    vector.tensor_scalar(
                        out=M[:, sb, 0, j, :], in0=GX1N, scalar1=nax1,
                        scalar2=None, op0=Alu.min,
                    )
                    vector.tensor_scalar(
                        out=M[:, sb, 1, j, :], in0=GY1N, scalar1=nay1,
                        scalar2=None, op0=Alu.min,
                    )
                # S slot reuse: ACT SXR(s-NB) and pool int(s-NB) must be done
                if s >= NB:
                    vector.wait_ge(act_sem, 2 * (s - NB) + 1)
                    vector.wait_ge(pool_sem, 2 * (s - NB) + 1)
                vector.tensor_tensor(
                    out=S[:, sb, :, :, :], in0=T[:, sb, :, :, :],
                    in1=M[:, sb, :, :, :], op=Alu.add,
                ).then_inc(dve_sem, 1)
                if s >= 2:
                    dve_mchain(vector, s - 2)
            dve_mchain(vector, NSB - 2)
            dve_mchain(vector, NSB - 1)
            # final: v = relu(vout); iou = v * (1 / (1 - v))
            vector.tensor_scalar(
                out=V2[:, :], in0=VOUT[:, :], scalar1=0.0, scalar2=None, op0=Alu.max
            ).then_inc(dve_sem, 1)
            vector.wait_ge(act_sem, 2 * NSB + 1)
            vector.tensor_tensor(
                out=MIOU[:, :], in0=V2[:, :], in1=R1[:, :], op=Alu.mult
            ).then_inc(dve_sem, 1)

        @block.scalar
        def _(scalar):
            scalar.wait_ge(dma_sem, 48)
            for s in range(NSB):
                sb = s % NB
                scalar.wait_ge(dve_sem, s + 1)
                if s >= NB:
                    scalar.wait_ge(pool_sem, 2 * (s - NB) + 1)
                scalar.activation(
                    out=SXR[:, sb, :, :], in_=S[:, sb, 0, :, :],
                    func=mybir.ActivationFunctionType.Relu,
                ).then_inc(act_sem, 1)
                for j in range(C):
                    blk = s * C + j
                    scalar.activation(
                        out=SA[:, sb, j, :], in_=GAREA,
                        func=mybir.ActivationFunctionType.Identity,
                        bias=AR[:, blk : blk + 1], scale=1.0,
                    )
                if s >= NB:
                    scalar.wait_ge(pool_sem, 2 * (s - NB) + 2)
                _act_recip(
                    scalar, nc, RS[:, sb, :, :], SA[:, sb, :, :]
                ).then_inc(act_sem, 1)
            scalar.wait_ge(dve_sem, NSB + 1)
            _act_recip(
                scalar, nc, R1[:, :], V2[:, :], bias=1.0, scale=-1.0
            ).then_inc(act_sem, 1)

        @block.gpsimd
        def _(gpsimd):
            for s in range(NSB):
                sb = s % NB
                gpsimd.wait_ge(act_sem, 2 * s + 1)
                gpsimd.tensor_tensor(
                    out=INT[:, sb, :, :], in0=SXR[:, sb, :, :],
                    in1=S[:, sb, 1, :, :], op=Alu.mult,
                ).then_inc(pool_sem, 1)
                gpsimd.wait_ge(act_sem, 2 * s + 2)
                gpsimd.tensor_tensor(
                    out=W[:, s % 3, :, :],
                    in0=INT[:, sb, :, :].rearrange("p c (bt g) -> p (c bt) g", bt=BATCH),
                    in1=RS[:, sb, :, :].rearrange("p c (bt g) -> p (c bt) g", bt=BATCH),
                    op=Alu.mult,
                ).then_inc(pool_sem, 1)

        @block.sync
        def _(sync):
            sync.wait_ge(dve_sem, NSB + 2)
            sync.dma_start(out=out_ext[:, :], in_=MIOU[:, :]).then_inc(dma_sem, 16)
            sync.wait_ge(dma_sem, 64)

    return nc


def kernel(anchors: np.ndarray, gt_boxes: np.ndarray) -> np.ndarray:
    global LAST_EXEC_NS
    anchors = np.asarray(anchors, dtype=np.float32) * COORD_SCALE
    gt_boxes = np.asarray(gt_boxes, dtype=np.float32) * COORD_SCALE

    apad = np.zeros((N_PAD, 4), dtype=np.float32)
    apad[:N_ANCHORS] = anchors

    g = gt_boxes.reshape(NPAIR, 4).astype(np.float32)
    garea = (g[:, 2] - g[:, 0]) * (g[:, 3] - g[:, 1])
    gtrows = np.stack([g[:, 2], -g[:, 0], g[:, 3], -g[:, 1], garea])
    gtrows = np.ascontiguousarray(gtrows.astype(np.float16))

    in_maps = []
    for c in range(N_CORES):
        sh = apad[c * N_LOC : (c + 1) * N_LOC]
        a3 = sh.reshape(P, BLOCKS, 4)
        asc = np.empty_like(a3)
        asc[:, :, 0] = a3[:, :, 2]   # ax2
        asc[:, :, 1] = -a3[:, :, 0]  # -ax1
        asc[:, :, 2] = a3[:, :, 3]   # ay2
        asc[:, :, 3] = -a3[:, :, 1]  # -ay1
        aarea = (a3[:, :, 2] - a3[:, :, 0]) * (a3[:, :, 3] - a3[:, :, 1])
        in_maps.append(
            {
                "ascal": np.ascontiguousarray(asc.reshape(P, BLOCKS * 4)),
                "aarea": np.ascontiguousarray(aarea.astype(np.float32)),
                "gtrows": gtrows,
            }
        )

    nc = _build_graph()
    trace = os.environ.get("ANCHOR_TRACE", "0") == "1"
    core_ids = list(range(N_CORES))
    if trace:
        _ensure_axon_ntff_hook()
        _patch_upload_artifacts()
        try:
            res = run_bass_kernel_spmd(nc, in_maps, core_ids=core_ids, trace=True)
        except Exception as e:
            print(f"trace run failed ({type(e).__name__}: {e}); falling back", file=sys.stderr)
            res = run_bass_kernel_spmd(nc, in_maps, core_ids=core_ids, trace=False)
    else:
        res = run_bass_kernel_spmd(nc, in_maps, core_ids=core_ids, trace=False)
    LAST_EXEC_NS = res.exec_time_ns

    out = np.empty((BATCH, N_PAD), dtype=np.float32)
    for c in range(N_CORES):
        o = res.results[c]["out"].reshape(P, BLOCKS, 4)
        out[:, c * N_LOC : (c + 1) * N_LOC] = o.transpose(2, 0, 1).reshape(BATCH, N_LOC)
    return out[:, :N_ANCHORS]


# revision 12
# speedup vs baseline: 1.3355x; 1.3355x over previous
"""AnchorTargetLayer max-IoU kernel for 8 TRN2 NeuronCores (v5, fp16).

max_iou[b, n] = max_g IoU(anchor_n, gt_box[b, g]);
anchors [100000, 4] f32, gt_boxes [4, 64, 4] f32 -> out [4, 100000] f32.

Sharding: anchors split 8 ways (12544/core incl pad), gt replicated, no
collectives. Per-core layout: anchors on SBUF partitions (128/block, 98
blocks), all B*G = 256 (batch, gt) pairs on the free dim, batch-major.

Coordinates pre-scaled by 1/16 on the host, GT rows cast to fp16 (anchor
per-partition scalars stay f32 as the ISA requires); chain err ~3.5e-3
vs the 2e-2 budget.

Engine facts this schedule is built on (measured on HW):
 - DVE tensor_scalar with an SBUF scalar hits the 4x fp16 perf mode
   (~275ns at 256 wide); scalar_tensor_tensor has NO fast uop (1x).
 - GpSimd (Pool) shares an SBUF port with DVE: any Pool op stalls
   concurrent 2-port DVE ops to Pool's (slow, 2.5cyc/elem) duration, so
   Pool is left COMPLETELY IDLE.
 - ACT (Scalar) runs independently; narrow ops cost ~224cyc bubble.

Per (anchor, pair), fp16:
  t = min(g2, a2), m = min(-g1, -a1)   4 TS-ptr (4x) per block, x|y
  s = t + m                            wide DVE TT 2x (both axes at once)
  sxr = relu(s_x)                      ACT wide relu
  int = sxr * s_y                      wide DVE TT (s_y relu deferred)
  sa  = garea + areaA                  ACT Identity+bias narrow
  rs  = 1/sa                           ACT Reciprocal wide
  w   = int * rs                       wide DVE TT
  red = max over g:64->32->16 pairwise DVE TT + tensor_reduce over 16
Final: v = relu(vout); iou = v / (1 - v).

Superblocks of C=14 blocks; S/RS double-buffered across DVE<->ACT; the
consumer chain for superblock s-1 runs on DVE after front(s) so ACT has
a full superblock of latency slack.
"""

import os
import sys

import numpy as np

sys.path.insert(0, "/opt/trn_rl_repo")

import concourse.bass as bass
import concourse.mybir as mybir
from concourse.bass_utils import run_bass_kernel_spmd

N_ANCHORS = 100000
BATCH = 4
N_GT = 64
N_CORES = 8

P = 128
BLOCKS = 98
C = 14                      # blocks per superblock
NSB = BLOCKS // C           # 7 superblocks
N_LOC = P * BLOCKS          # 12544
N_PAD = N_LOC * N_CORES     # 100352
NPAIR = BATCH * N_GT        # 256
SEG = C * BATCH             # 56 (block, batch) rows per superblock
NB = 2                      # S/RS double-buffer depth

F32 = mybir.dt.float32
F16 = mybir.dt.float16
COORD_SCALE = 1.0 / 16.0

LAST_EXEC_NS = None


def _ensure_axon_ntff_hook():
    try:
        import antenv.axon_hooks  # noqa: F401

        return
    except ImportError:
        pass
    import contextlib
    import ctypes
    import types

    import antenv

    m = types.ModuleType("antenv.axon_hooks")
    m._hook = None

    def set_axon_ntff_profile_hook(h):
        m._hook = h

    def get_axon_ntff_profile_hook():
        return m._hook

    m.set_axon_ntff_profile_hook = set_axon_ntff_profile_hook
    m.get_axon_ntff_profile_hook = get_axon_ntff_profile_hook
    sys.modules["antenv.axon_hooks"] = m
    antenv.axon_hooks = m

    so_path = os.environ.get("PJRT_LIBRARY_PATH", "/opt/axon/libaxon_pjrt.so")
    try:
        lib = ctypes.CDLL(so_path)
    except OSError:
        return
    if not hasattr(lib, "axon_start_nrt_profile"):
        return
    lib.axon_start_nrt_profile.argtypes = [
        ctypes.POINTER(ctypes.c_int64),
        ctypes.c_size_t,
    ]
    lib.axon_start_nrt_profile.restype = ctypes.c_int64
    lib.axon_stop_nrt_profile.argtypes = [ctypes.c_char_p]
    lib.axon_stop_nrt_profile.restype = ctypes.c_int64

    @contextlib.contextmanager
    def _hook(output_dir, device_ids):
        import jax

        jax.devices()
        if device_ids:
            ids = (ctypes.c_int64 * len(device_ids))(*device_ids)
            rc = lib.axon_start_nrt_profile(ids, len(device_ids))
        else:
            rc = lib.axon_start_nrt_profile(None, 0)
        if rc != 0:
            raise RuntimeError(f"axon_start_nrt_profile rc={rc}")
        try:
            yield
        finally:
            n = lib.axon_stop_nrt_profile(str(output_dir).encode())
            if n < 0:
                raise RuntimeError(f"axon_stop_nrt_profile rc={n}")

    set_axon_ntff_profile_hook(_hook)


def _patch_upload_artifacts():
    import concourse.bass_utils as bu

    if getattr(bu.upload_artifacts, "_safe", False):
        return
    orig = bu.upload_artifacts

    def safe(tmpdir):
        try:
            return orig(tmpdir)
        except Exception:
            return tmpdir

    safe._safe = True
    bu.upload_artifacts = safe


def _act_recip(scalar_eng, nc, out_ap, in_ap, bias=0.0, scale=1.0):
    """Directly emit Activation(Reciprocal) (the nc.scalar.activation wrapper
    rejects Reciprocal)."""
    ins = [scalar_eng.lower_ap(in_ap)]
    for argv in (bias, scale, 0.0):  # bias, scale, alpha
        ins.append(mybir.ImmediateValue(dtype=F32, value=argv))
    return scalar_eng.add_instruction(
        mybir.InstActivation(
            name=nc.get_next_instruction_name(),
            func=mybir.ActivationFunctionType.Reciprocal,
            ins=ins,
            outs=[scalar_eng.lower_ap(out_ap)],
        )
    )


def _build_graph():
    nc = bass.Bass()
    ASC_ext = nc.declare_dram_parameter("ascal", [P, BLOCKS * 4], F32, isOutput=False)
    AR_ext = nc.declare_dram_parameter("aarea", [P, BLOCKS], F32, isOutput=False)
    GT_ext = nc.declare_dram_parameter("gtrows", [5, NPAIR], F16, isOutput=False)
    out_ext = nc.declare_dram_parameter("out", [P, BLOCKS * 4], F32, isOutput=True)

    Alu = mybir.AluOpType
    from contextlib import ExitStack

    with ExitStack() as _st:
        e = _st.enter_context
        ASC = e(nc.sbuf_tensor("ASC", [P, BLOCKS * 4], F32))
        AR = e(nc.sbuf_tensor("AR", [P, BLOCKS], F32))
        GTB = e(nc.sbuf_tensor("GTB", [P, 5, NPAIR], F16))
        T = e(nc.sbuf_tensor("T", [P, 2, C, NPAIR], F16))
        M = e(nc.sbuf_tensor("M", [P, 2, C, NPAIR], F16))
        S = e(nc.sbuf_tensor("S", [P, NB, 2, C, NPAIR], F16))
        SXR = e(nc.sbuf_tensor("SXR", [P, NB, C, NPAIR], F16))
        INT = e(nc.sbuf_tensor("INT", [P, C, NPAIR], F16))
        SA = e(nc.sbuf_tensor("SA", [P, C, NPAIR], F16))
        RS = e(nc.sbuf_tensor("RS", [P, NB, C, NPAIR], F16))
        W = e(nc.sbuf_tensor("W", [P, SEG, N_GT], F16))
        M1 = e(nc.sbuf_tensor("M1", [P, SEG, 32], F16))
        M2 = e(nc.sbuf_tensor("M2", [P, SEG, 16], F16))
        VOUT = e(nc.sbuf_tensor("VOUT", [P, BLOCKS * 4], F32))
        V2 = e(nc.sbuf_tensor("V2", [P, BLOCKS * 4], F32))
        R1 = e(nc.sbuf_tensor("R1", [P, BLOCKS * 4], F32))
        MIOU = e(nc.sbuf_tensor("MIOU", [P, BLOCKS * 4], F32))
        block = e(nc.Block())
        dma_sem = e(nc.semaphore("dma_sem"))
        dve_sem = e(nc.semaphore("dve_sem"))
        act_sem = e(nc.semaphore("act_sem"))

        GX2 = GTB[:, 0, :]
        GX1N = GTB[:, 1, :]
        GY2 = GTB[:, 2, :]
        GY1N = GTB[:, 3, :]
        GAREA = GTB[:, 4, :]

        @block.sync
        def _(sync):
            sync.dma_start(out=ASC[:, :], in_=ASC_ext[:, :]).then_inc(dma_sem, 16)
            sync.dma_start(out=AR[:, :], in_=AR_ext[:, :]).then_inc(dma_sem, 16)
            g_ap = GT_ext[:, :]
            g_b = bass.AP(
                tensor=g_ap.tensor, offset=g_ap.offset, ap=[[0, P]] + list(g_ap.ap)
            )
            sync.dma_start(out=GTB[:, :, :], in_=g_b).then_inc(dma_sem, 16)

        # sem targets: dve: s+1 after S(s); NSB+1 v2; NSB+2 miou
        #              act: 2s+1 sxr(s); 2s+2 rs(s); 2NSB+1 r1
        def dve_tail(vector, s):
            """Consumer chain for superblock s: int, w, pairwise maxes,
            reduce. Runs on DVE one superblock behind the front."""
            sb = s % NB
            vector.wait_ge(act_sem, 2 * s + 1)
            vector.tensor_tensor(
                out=INT[:, :, :], in0=SXR[:, sb, :, :], in1=S[:, sb, 1, :, :],
                op=Alu.mult,
            )
            vector.wait_ge(act_sem, 2 * s + 2)
            vector.tensor_tensor(
                out=W[:, :, :],
                in0=INT[:, :, :].rearrange("p c (bt g) -> p (c bt) g", bt=BATCH),
                in1=RS[:, sb, :, :].rearrange("p c (bt g) -> p (c bt) g", bt=BATCH),
                op=Alu.mult,
            )
            vector.tensor_tensor(
                out=M1[:, :, :], in0=W[:, :, 0:32], in1=W[:, :, 32:64], op=Alu.max
            )
            vector.tensor_tensor(
                out=M2[:, :, :], in0=M1[:, :, 0:16], in1=M1[:, :, 16:32], op=Alu.max
            )
            vector.tensor_reduce(
                out=VOUT[:, s * SEG : (s + 1) * SEG],
                in_=M2[:, :, :],
                axis=mybir.AxisListType.X,
                op=Alu.max,
            )

        @block.vector
        def _(vector):
            vector.wait_ge(dma_sem, 48)
            for s in range(NSB):
                sb = s % NB
                for j in range(C):
                    blk = s * C + j
                    ax2 = ASC[:, 4 * blk + 0 : 4 * blk + 1]
                    nax1 = ASC[:, 4 * blk + 1 : 4 * blk + 2]
                    ay2 = ASC[:, 4 * blk + 2 : 4 * blk + 3]
                    nay1 = ASC[:, 4 * blk + 3 : 4 * blk + 4]
                    vector.tensor_scalar(
                        out=T[:, 0, j, :], in0=GX2, scalar1=ax2,
                        scalar2=None, op0=Alu.min,
                    )
                    vector.tensor_scalar(
                        out=T[:, 1, j, :], in0=GY2, scalar1=ay2,
                        scalar2=None, op0=Alu.min,
                    )
                    vector.tensor_scalar(
                        out=M[:, 0, j, :], in0=GX1N, scalar1=nax1,
                        scalar2=None, op0=Alu.min,
                    )
                    vector.tensor_scalar(
                        out=M[:, 1, j, :], in0=GY1N, scalar1=nay1,
                        scalar2=None, op0=Alu.min,
                    )
                if s >= 1:
                    dve_tail(vector, s - 1)
                # S slot: ACT sxr(s-NB) must have read it (int(s-NB) already
                # done on this engine via the tail above)
                if s >= NB:
                    vector.wait_ge(act_sem, 2 * (s - NB) + 1)
                vector.tensor_tensor(
                    out=S[:, sb, :, :, :], in0=T[:, :, :, :], in1=M[:, :, :, :],
                    op=Alu.add,
                ).then_inc(dve_sem, 1)
            dve_tail(vector, NSB - 1)
            # final: v = relu(vout); iou = v * (1 / (1 - v))
            vector.tensor_scalar(
                out=V2[:, :], in0=VOUT[:, :], scalar1=0.0, scalar2=None, op0=Alu.max
            ).then_inc(dve_sem, 1)
            vector.wait_ge(act_sem, 2 * NSB + 1)
            vector.tensor_tensor(
                out=MIOU[:, :], in0=V2[:, :], in1=R1[:, :], op=Alu.mult
            ).then_inc(dve_sem, 1)

        @block.scalar
        def _(scalar):
            scalar.wait_ge(dma_sem, 48)
            for s in range(NSB):
                sb = s % NB
                # dve >= s+1 means S(s) done; it also means int(s-2)/w(s-2)
                # are done (they precede S(s) on DVE), freeing SXR/RS slots
                scalar.wait_ge(dve_sem, s + 1)
                scalar.activation(
                    out=SXR[:, sb, :, :], in_=S[:, sb, 0, :, :],
                    func=mybir.ActivationFunctionType.Relu,
                ).then_inc(act_sem, 1)
                for j in range(C):
                    blk = s * C + j
                    scalar.activation(
                        out=SA[:, j, :], in_=GAREA,
                        func=mybir.ActivationFunctionType.Identity,
                        bias=AR[:, blk : blk + 1], scale=1.0,
                    )
                _act_recip(
                    scalar, nc, RS[:, sb, :, :], SA[:, :, :]
                ).then_inc(act_sem, 1)
            scalar.wait_ge(dve_sem, NSB + 1)
            _act_recip(
                scalar, nc, R1[:, :], V2[:, :], bias=1.0, scale=-1.0
            ).then_inc(act_sem, 1)

        @block.sync
        def _(sync):
            sync.wait_ge(dve_sem, NSB + 2)
            sync.dma_start(out=out_ext[:, :], in_=MIOU[:, :]).then_inc(dma_sem, 16)
            sync.wait_ge(dma_sem, 64)

    return nc


def kernel(anchors: np.ndarray, gt_boxes: np.ndarray) -> np.ndarray:
    global LAST_EXEC_NS
    anchors = np.asarray(anchors, dtype=np.float32) * COORD_SCALE
    gt_boxes = np.asarray(gt_boxes, dtype=np.float32) * COORD_SCALE

    apad = np.zeros((N_PAD, 4), dtype=np.float32)
    apad[:N_ANCHORS] = anchors

    g = gt_boxes.reshape(NPAIR, 4).astype(np.float32)
    garea = (g[:, 2] - g[:, 0]) * (g[:, 3] - g[:, 1])
    gtrows = np.stack([g[:, 2], -g[:, 0], g[:, 3], -g[:, 1], garea])
    gtrows = np.ascontiguousarray(gtrows.astype(np.float16))

    in_maps = []
    for c in range(N_CORES):
        sh = apad[c * N_LOC : (c + 1) * N_LOC]
        a3 = sh.reshape(P, BLOCKS, 4)
        asc = np.empty_like(a3)
        asc[:, :, 0] = a3[:, :, 2]   # ax2
        asc[:, :, 1] = -a3[:, :, 0]  # -ax1
        asc[:, :, 2] = a3[:, :, 3]   # ay2
        asc[:, :, 3] = -a3[:, :, 1]  # -ay1
        aarea = (a3[:, :, 2] - a3[:, :, 0]) * (a3[:, :, 3] - a3[:, :, 1])
        in_maps.append(
            {
                "ascal": np.ascontiguousarray(asc.reshape(P, BLOCKS * 4)),
                "aarea": np.ascontiguousarray(aarea.astype(np.float32)),
                "gtrows": gtrows,
            }
        )

    nc = _build_graph()
    trace = os.environ.get("ANCHOR_TRACE", "0") == "1"
    core_ids = list(range(N_CORES))
    if trace:
        _ensure_axon_ntff_hook()
        _patch_upload_artifacts()
        try:
            res = run_bass_kernel_spmd(nc, in_maps, core_ids=core_ids, trace=True)
        except Exception as e:
            print(f"trace run failed ({type(e).__name__}: {e}); falling back", file=sys.stderr)
            res = run_bass_kernel_spmd(nc, in_maps, core_ids=core_ids, trace=False)
    else:
        res = run_bass_kernel_spmd(nc, in_maps, core_ids=core_ids, trace=False)
    LAST_EXEC_NS = res.exec_time_ns

    out = np.empty((BATCH, N_PAD), dtype=np.float32)
    for c in range(N_CORES):
        o = res.results[c]["out"].reshape(P, BLOCKS, 4)
        out[:, c * N_LOC : (c + 1) * N_LOC] = o.transpose(2, 0, 1).reshape(BATCH, N_LOC)
    return out[:, :N_ANCHORS]


# revision 16
# speedup vs baseline: 1.4172x; 1.0612x over previous
"""AnchorTargetLayer max-IoU kernel for 8 TRN2 NeuronCores (v5, fp16).

max_iou[b, n] = max_g IoU(anchor_n, gt_box[b, g]);
anchors [100000, 4] f32, gt_boxes [4, 64, 4] f32 -> out [4, 100000] f32.

Sharding: anchors split 8 ways (12544/core incl pad), gt replicated, no
collectives. Per-core layout: anchors on SBUF partitions (128/block, 98
blocks), all B*G = 256 (batch, gt) pairs on the free dim, batch-major.

Coordinates pre-scaled by 1/16 on the host, GT rows cast to fp16 (anchor
per-partition scalars stay f32 as the ISA requires); chain err ~3.5e-3
vs the 2e-2 budget.

Engine facts this schedule is built on (measured on HW):
 - DVE tensor_scalar with an SBUF scalar hits the 4x fp16 perf mode
   (~275ns at 256 wide); scalar_tensor_tensor has NO fast uop (1x).
 - GpSimd (Pool) shares an SBUF port with DVE: any Pool op stalls
   concurrent 2-port DVE ops to Pool's (slow, 2.5cyc/elem) duration, so
   Pool is left COMPLETELY IDLE.
 - ACT (Scalar) runs independently; narrow ops cost ~224cyc bubble.

Per (anchor, pair), fp16:
  t = min(g2, a2), m = min(-g1, -a1)   4 TS-ptr (4x) per block, x|y
  s = t + m                            wide DVE TT 2x (both axes at once)
  sxr = relu(s_x)                      ACT wide relu
  int = sxr * s_y                      wide DVE TT (s_y relu deferred)
  sa  = garea + areaA                  ACT Identity+bias narrow
  rs  = 1/sa                           ACT Reciprocal wide
  w   = int * rs                       wide DVE TT
  red = max over g:64->32->16 pairwise DVE TT + tensor_reduce over 16
Final: v = relu(vout); iou = v / (1 - v).

Superblocks of C=14 blocks; S/RS double-buffered across DVE<->ACT; the
consumer chain for superblock s-1 runs on DVE after front(s) so ACT has
a full superblock of latency slack.
"""

import os
import sys

import numpy as np

sys.path.insert(0, "/opt/trn_rl_repo")

import concourse.bass as bass
import concourse.mybir as mybir
from concourse.bass_utils import run_bass_kernel_spmd

N_ANCHORS = 100000
BATCH = 4
N_GT = 64
N_CORES = 8

P = 128
BLOCKS = 98
C = 14                      # blocks per superblock
NSB = BLOCKS // C           # 7 superblocks
N_LOC = P * BLOCKS          # 12544
N_PAD = N_LOC * N_CORES     # 100352
NPAIR = BATCH * N_GT        # 256
SEG = C * BATCH             # 56 (block, batch) rows per superblock
NB = 2                      # S/RS double-buffer depth

F32 = mybir.dt.float32
F16 = mybir.dt.float16
COORD_SCALE = 1.0 / 16.0

LAST_EXEC_NS = None


def _ensure_axon_ntff_hook():
    try:
        import antenv.axon_hooks  # noqa: F401

        return
    except ImportError:
        pass
    import contextlib
    import ctypes
    import types

    import antenv

    m = types.ModuleType("antenv.axon_hooks")
    m._hook = None

    def set_axon_ntff_profile_hook(h):
        m._hook = h

    def get_axon_ntff_profile_hook():
        return m._hook

    m.set_axon_ntff_profile_hook = set_axon_ntff_profile_hook
    m.get_axon_ntff_profile_hook = get_axon_ntff_profile_hook
    sys.modules["antenv.axon_hooks"] = m
    antenv.axon_hooks = m

    so_path = os.environ.get("PJRT_LIBRARY_PATH", "/opt/axon/libaxon_pjrt.so")
    try:
        lib = ctypes.CDLL(so_path)
    except OSError:
        return
    if not hasattr(lib, "axon_start_nrt_profile"):
        return
    lib.axon_start_nrt_profile.argtypes = [
        ctypes.POINTER(ctypes.c_int64),
        ctypes.c_size_t,
    ]
    lib.axon_start_nrt_profile.restype = ctypes.c_int64
    lib.axon_stop_nrt_profile.argtypes = [ctypes.c_char_p]
    lib.axon_stop_nrt_profile.restype = ctypes.c_int64

    @contextlib.contextmanager
    def _hook(output_dir, device_ids):
        import jax

        jax.devices()
        if device_ids:
            ids = (ctypes.c_int64 * len(device_ids))(*device_ids)
            rc = lib.axon_start_nrt_profile(ids, len(device_ids))
        else:
            rc = lib.axon_start_nrt_profile(None, 0)
        if rc != 0:
            raise RuntimeError(f"axon_start_nrt_profile rc={rc}")
        try:
            yield
        finally:
            n = lib.axon_stop_nrt_profile(str(output_dir).encode())
            if n < 0:
                raise RuntimeError(f"axon_stop_nrt_profile rc={n}")

    set_axon_ntff_profile_hook(_hook)


def _patch_upload_artifacts():
    import concourse.bass_utils as bu

    if getattr(bu.upload_artifacts, "_safe", False):
        return
    orig = bu.upload_artifacts

    def safe(tmpdir):
        try:
            return orig(tmpdir)
        except Exception:
            return tmpdir

    safe._safe = True
    bu.upload_artifacts = safe


def _act_recip(scalar_eng, nc, out_ap, in_ap, bias=0.0, scale=1.0):
    """Directly emit Activation(Reciprocal) (the nc.scalar.activation wrapper
    rejects Reciprocal)."""
    ins = [scalar_eng.lower_ap(in_ap)]
    for argv in (bias, scale, 0.0):  # bias, scale, alpha
        ins.append(mybir.ImmediateValue(dtype=F32, value=argv))
    return scalar_eng.add_instruction(
        mybir.InstActivation(
            name=nc.get_next_instruction_name(),
            func=mybir.ActivationFunctionType.Reciprocal,
            ins=ins,
            outs=[scalar_eng.lower_ap(out_ap)],
        )
    )


def _build_graph():
    nc = bass.Bass()
    ASC_ext = nc.declare_dram_parameter("ascal", [P, BLOCKS * 4], F32, isOutput=False)
    AR_ext = nc.declare_dram_parameter("aarea", [P, BLOCKS], F32, isOutput=False)
    GT_ext = nc.declare_dram_parameter("gtrows", [5, NPAIR], F16, isOutput=False)
    out_ext = nc.declare_dram_parameter("out", [P, BLOCKS * 4], F32, isOutput=True)

    Alu = mybir.AluOpType
    from contextlib import ExitStack

    with ExitStack() as _st:
        e = _st.enter_context
        ASC = e(nc.sbuf_tensor("ASC", [P, BLOCKS * 4], F32))
        AR = e(nc.sbuf_tensor("AR", [P, BLOCKS], F32))
        GTB = e(nc.sbuf_tensor("GTB", [P, 5, NPAIR], F16))
        T = e(nc.sbuf_tensor("T", [P, 2, C, NPAIR], F16))
        M = e(nc.sbuf_tensor("M", [P, 2, C, NPAIR], F16))
        S = e(nc.sbuf_tensor("S", [P, NB, 2, C, NPAIR], F16))
        SXR = e(nc.sbuf_tensor("SXR", [P, NB, C, NPAIR], F16))
        INT = e(nc.sbuf_tensor("INT", [P, C, NPAIR], F16))
        SA = e(nc.sbuf_tensor("SA", [P, NB, C, NPAIR], F16))
        RS = e(nc.sbuf_tensor("RS", [P, NB, C, NPAIR], F16))
        W = e(nc.sbuf_tensor("W", [P, SEG, N_GT], F16))
        M1 = e(nc.sbuf_tensor("M1", [P, SEG, 32], F16))
        M2 = e(nc.sbuf_tensor("M2", [P, SEG, 16], F16))
        VOUT = e(nc.sbuf_tensor("VOUT", [P, BLOCKS * 4], F32))
        V2 = e(nc.sbuf_tensor("V2", [P, BLOCKS * 4], F32))
        R1 = e(nc.sbuf_tensor("R1", [P, BLOCKS * 4], F32))
        MIOU = e(nc.sbuf_tensor("MIOU", [P, BLOCKS * 4], F32))
        block = e(nc.Block())
        dma_sem = e(nc.semaphore("dma_sem"))
        dve_sem = e(nc.semaphore("dve_sem"))
        act_sem = e(nc.semaphore("act_sem"))

        GX2 = GTB[:, 0, :]
        GX1N = GTB[:, 1, :]
        GY2 = GTB[:, 2, :]
        GY1N = GTB[:, 3, :]
        GAREA = GTB[:, 4, :]

        @block.sync
        def _(sync):
            sync.dma_start(out=ASC[:, :], in_=ASC_ext[:, :]).then_inc(dma_sem, 16)
            sync.dma_start(out=AR[:, :], in_=AR_ext[:, :]).then_inc(dma_sem, 16)
            g_ap = GT_ext[:, :]
            g_b = bass.AP(
                tensor=g_ap.tensor, offset=g_ap.offset, ap=[[0, P]] + list(g_ap.ap)
            )
            sync.dma_start(out=GTB[:, :, :], in_=g_b).then_inc(dma_sem, 16)

        # sem targets: dve: s+1 after S(s); NSB+1 v2; NSB+2 miou
        #              act: 2s+1 sxr(s); 2s+2 rs(s); 2NSB+1 r1
        def dve_tail(vector, s, self_sxr=False):
            """Consumer chain for superblock s: int, w, pairwise maxes,
            reduce. Runs on DVE one superblock behind the front (except the
            last superblock, which computes its own relu to avoid waiting
            for ACT)."""
            sb = s % NB
            if self_sxr:
                vector.tensor_scalar(
                    out=SXR[:, sb, :, :], in0=S[:, sb, 0, :, :], scalar1=0.0,
                    scalar2=None, op0=Alu.max,
                )
            else:
                vector.wait_ge(act_sem, 2 * s + 2)  # sxr(s)
            vector.tensor_tensor(
                out=INT[:, :, :], in0=SXR[:, sb, :, :], in1=S[:, sb, 1, :, :],
                op=Alu.mult,
            )
            vector.wait_ge(act_sem, 2 * s + 1)  # rs(s)
            vector.tensor_tensor(
                out=W[:, :, :],
                in0=INT[:, :, :].rearrange("p c (bt g) -> p (c bt) g", bt=BATCH),
                in1=RS[:, sb, :, :].rearrange("p c (bt g) -> p (c bt) g", bt=BATCH),
                op=Alu.mult,
            )
            vector.tensor_tensor(
                out=M1[:, :, :], in0=W[:, :, 0:32], in1=W[:, :, 32:64], op=Alu.max
            )
            vector.tensor_tensor(
                out=M2[:, :, :], in0=M1[:, :, 0:16], in1=M1[:, :, 16:32], op=Alu.max
            )
            vector.tensor_reduce(
                out=VOUT[:, s * SEG : (s + 1) * SEG],
                in_=M2[:, :, :],
                axis=mybir.AxisListType.X,
                op=Alu.max,
            )

        @block.vector
        def _(vector):
            vector.wait_ge(dma_sem, 48)
            for s in range(NSB):
                sb = s % NB
                for j in range(C):
                    blk = s * C + j
                    ax2 = ASC[:, 4 * blk + 0 : 4 * blk + 1]
                    nax1 = ASC[:, 4 * blk + 1 : 4 * blk + 2]
                    ay2 = ASC[:, 4 * blk + 2 : 4 * blk + 3]
                    nay1 = ASC[:, 4 * blk + 3 : 4 * blk + 4]
                    vector.tensor_scalar(
                        out=T[:, 0, j, :], in0=GX2, scalar1=ax2,
                        scalar2=None, op0=Alu.min,
                    )
                    vector.tensor_scalar(
                        out=T[:, 1, j, :], in0=GY2, scalar1=ay2,
                        scalar2=None, op0=Alu.min,
                    )
                    vector.tensor_scalar(
                        out=M[:, 0, j, :], in0=GX1N, scalar1=nax1,
                        scalar2=None, op0=Alu.min,
                    )
                    vector.tensor_scalar(
                        out=M[:, 1, j, :], in0=GY1N, scalar1=nay1,
                        scalar2=None, op0=Alu.min,
                    )
                if s >= 1:
                    dve_tail(vector, s - 1)
                # S slot: ACT sxr(s-NB) must have read it (int(s-NB) already
                # done on this engine via the tail above)
                if s >= NB:
                    vector.wait_ge(act_sem, 2 * (s - NB) + 2)
                vector.tensor_tensor(
                    out=S[:, sb, :, :, :], in0=T[:, :, :, :], in1=M[:, :, :, :],
                    op=Alu.add,
                ).then_inc(dve_sem, 1)
            dve_tail(vector, NSB - 1, self_sxr=True)
            # final: v = relu(vout); iou = v * (1 / (1 - v))
            vector.tensor_scalar(
                out=V2[:, :], in0=VOUT[:, :], scalar1=0.0, scalar2=None, op0=Alu.max
            ).then_inc(dve_sem, 1)
            vector.wait_ge(act_sem, 2 * NSB + 1)
            vector.tensor_tensor(
                out=MIOU[:, :], in0=V2[:, :], in1=R1[:, :], op=Alu.mult
            ).then_inc(dve_sem, 1)

        @block.scalar
        def _(scalar):
            scalar.wait_ge(dma_sem, 48)

            def sa_batch(s):
                for j in range(C):
                    blk = s * C + j
                    scalar.activation(
                        out=SA[:, s % NB, j, :], in_=GAREA,
                        func=mybir.ActivationFunctionType.Identity,
                        bias=AR[:, blk : blk + 1], scale=1.0,
                    )

            sa_batch(0)
            for s in range(NSB):
                sb = s % NB
                # rs(s) only needs SA(s) (done an iteration ago); dve >= s
                # means w(s-2) has read the RS slot being overwritten
                if s >= NB:
                    scalar.wait_ge(dve_sem, s)
                _act_recip(
                    scalar, nc, RS[:, sb, :, :], SA[:, sb, :, :]
                ).then_inc(act_sem, 1)
                # dve >= s+1 means S(s) done; it also means int(s-2) is done
                # (it precedes S(s) on DVE), freeing the SXR slot
                if s + 1 < NSB:
                    scalar.wait_ge(dve_sem, s + 1)
                    scalar.activation(
                        out=SXR[:, sb, :, :], in_=S[:, sb, 0, :, :],
                        func=mybir.ActivationFunctionType.Relu,
                    ).then_inc(act_sem, 1)
                    sa_batch(s + 1)
                else:
                    # DVE computes its own relu for the last superblock;
                    # keep the act tick numbering with a tiny copy
                    scalar.activation(
                        out=SA[:, sb, 0, 0:1], in_=SA[:, sb, 0, 0:1],
                        func=mybir.ActivationFunctionType.Identity,
                    ).then_inc(act_sem, 1)
            scalar.wait_ge(dve_sem, NSB + 1)
            _act_recip(
                scalar, nc, R1[:, :], V2[:, :], bias=1.0, scale=-1.0
            ).then_inc(act_sem, 1)

        @block.sync
        def _(sync):
            sync.wait_ge(dve_sem, NSB + 2)
            sync.dma_start(out=out_ext[:, :], in_=MIOU[:, :]).then_inc(dma_sem, 16)
            sync.wait_ge(dma_sem, 64)

    return nc


def kernel(anchors: np.ndarray, gt_boxes: np.ndarray) -> np.ndarray:
    global LAST_EXEC_NS
    anchors = np.asarray(anchors, dtype=np.float32) * COORD_SCALE
    gt_boxes = np.asarray(gt_boxes, dtype=np.float32) * COORD_SCALE

    apad = np.zeros((N_PAD, 4), dtype=np.float32)
    apad[:N_ANCHORS] = anchors

    g = gt_boxes.reshape(NPAIR, 4).astype(np.float32)
    garea = (g[:, 2] - g[:, 0]) * (g[:, 3] - g[:, 1])
    gtrows = np.stack([g[:, 2], -g[:, 0], g[:, 3], -g[:, 1], garea])
    gtrows = np.ascontiguousarray(gtrows.astype(np.float16))

    in_maps = []
    for c in range(N_CORES):
        sh = apad[c * N_LOC : (c + 1) * N_LOC]
        a3 = sh.reshape(P, BLOCKS, 4)
        asc = np.empty_like(a3)
        asc[:, :, 0] = a3[:, :, 2]   # ax2
        asc[:, :, 1] = -a3[:, :, 0]  # -ax1
        asc[:, :, 2] = a3[:, :, 3]   # ay2
        asc[:, :, 3] = -a3[:, :, 1]  # -ay1
        aarea = (a3[:, :, 2] - a3[:, :, 0]) * (a3[:, :, 3] - a3[:, :, 1])
        in_maps.append(
            {
                "ascal": np.ascontiguousarray(asc.reshape(P, BLOCKS * 4)),
                "aarea": np.ascontiguousarray(aarea.astype(np.float32)),
                "gtrows": gtrows,
            }
        )

    nc = _build_graph()
    trace = os.environ.get("ANCHOR_TRACE", "0") == "1"
    core_ids = list(range(N_CORES))
    if trace:
        _ensure_axon_ntff_hook()
        _patch_upload_artifacts()
        try:
            res = run_bass_kernel_spmd(nc, in_maps, core_ids=core_ids, trace=True)
        except Exception as e:
            print(f"trace run failed ({type(e).__name__}: {e}); falling back", file=sys.stderr)
            res = run_bass_kernel_spmd(nc, in_maps, core_ids=core_ids, trace=False)
    else:
        res = run_bass_kernel_spmd(nc, in_maps, core_ids=core_ids, trace=False)
    LAST_EXEC_NS = res.exec_time_ns

    out = np.empty((BATCH, N_PAD), dtype=np.float32)
    for c in range(N_CORES):
        o = res.results[c]["out"].reshape(P, BLOCKS, 4)
        out[:, c * N_LOC : (c + 1) * N_LOC] = o.transpose(2, 0, 1).reshape(BATCH, N_LOC)
    return out[:, :N_ANCHORS]
